# revision 15
# baseline (speedup 1.0000x reference)
"""Bass/Trainium2 kernel for nn_Epdiff: Hermitian-truncated EPDiff smoothing
filters.

reference:
    cc(g) = -2*cos(2*pi*g) + 2
    s[i,j,k] = 3*(cc(gx)[i] + cc(gy)[j] + cc(gz)[k]) + 1     (gx,gy 2m-band, gz m)
    sharp  = s**6, smooth = s**-6, broadcast to [B, 1, 2m, 2m, m]

Work reduction before any device code runs:
  - The batch axis is a pure broadcast in the reference (jnp.broadcast_to):
    the unique output is one [128, 128, 64] plane per output.
  - cc is even around g=0.5, so the x band is mirror symmetric
    (plane[i] == plane[128-i] for i > 64) and likewise the y band:
    only the [65, 65, 64] corner block is unique (~1/3.9 of the plane).
The host unshard step expands batch with np.broadcast_to (zero copy) and the
x/y mirrors with two cheap np.take index maps (error vs the reference's
directly-evaluated cos values is ~1 ulp of cos, amplified to ~5e-6 relative).

The unique block ([65 partitions (x), 65*64 = 4160 free columns (y,z)]) is
sharded 520 columns per core.  Both outputs are evaluated entirely on the PE
as rank-structured matmuls over separable per-axis factors
(a_f = 3*(cc(gy)+cc(gz)), b_i = 3*cc(gx)+1, s = a + b):
  - sharp:  s^6 = sum_k C(6,k) a^k b^(6-k)           (7 binomial terms)
  - smooth: s^-6 ~= sum_n w_n e^(-t_n a) e^(-t_n b)  (10-node positive
    exponential-sum fit, abs err 1.7e-7 on s in [1,19]; the error metric is
    relative to max|smooth| = 1, so absolute accuracy is what matters)
Each factor is split into bf16 components (3-way / pairs p+q<=2 for sharp ->
42 rows; 2-way / 3 pairs for smooth -> 30 rows) so the bf16 PE reproduces the
f32 products to ~6e-7 / ~3e-6 while streaming 1 column/cycle.  The PE does
the partition broadcast for free: no SBUF broadcast fill, no ACT/DVE power
or reciprocal chain.

Scheduling: the NEFF entry/exit protocol costs ~6.7us + ~1.2us no matter
what, every DMA trigger occupies its queue ~0.6-1.2us plus ~1.2us of queue
kick latency, and a DMA-completion semaphore takes ~0.5-1us to land, so the
kernel minimizes DMA count and spreads the two fast HWDGE rings:
  sync  : sharp-factor fill -> combined chunk-0 output write -> retire
  scalar: smooth-factor fill -> sharp PSUM->SBUF copies (ACT Copy needs no
          table swap; its one-time table load hides under the fills)
          -> combined chunk-1 output write
  tensor: p-state warmup matmul + 4 real matmuls (2 chunks x 2 outputs)
  vector: smooth PSUM->SBUF copies, with spacer waits observing the ACT
          copies so each combined write needs only one semaphore wait
  gpsimd: unused (Q7 triggers are ~700ns and their semaphores lag ~2us)
Results are packed chunk-major in one SBUF tile ([65, 1040] =
sharp_c0|smooth_c0|sharp_c1|smooth_c1) so each write moves 65 x 2080-byte
lines (DMA packet-rate is ~170ns/packet/engine; fewer, bigger packets win);
the host splits the packing during unshard.
"""

import os
import numpy as np

# ---- problem constants (hardcoded per spec) ----
MODE = 64
TWO_M = 2 * MODE            # 128 = full x/y band size
XU = MODE + 1               # 65 unique x rows (partition axis)
YU = MODE + 1               # 65 unique y blocks
FREE_U = YU * MODE          # 4160 unique free columns (y,z)
BATCH = 32
N_CORES = 8
FREE_LOC = FREE_U // N_CORES  # 520 columns per core
CHUNK = FREE_LOC // 2       # 260 columns: fits one PSUM bank, <= 512 moving
N_CHUNKS = 2
KPOW = 7                    # binomial terms k = 0..6
PAIRS3 = ((0, 0), (0, 1), (1, 0), (0, 2), (1, 1), (2, 0))
PAIRS2 = ((0, 0), (0, 1), (1, 0))
K6 = KPOW * len(PAIRS3)     # 42 contraction rows for sharp
# positive exponential-sum nodes/weights for s^-6 on [1, 19]
# (bounded least squares on log-spaced nodes; abs err 1.7e-7)
EXP_T = (0.5, 0.7613351081512586, 1.1592622938073773, 1.7651741676630313,
         2.6877781316870837, 4.092599709149147, 6.231679684369749,
         9.488793052927289, 14.44830257035062, 22.0)
EXP_W = (0.00013164787718447632, 0.00024017744752813375,
         0.010191058868615295, 0.10040649134112098, 1.3406085839989488,
         16.376637870042973, 205.80671073439115, 2549.8737222649515,
         32029.378543336225, 466119.860099446)
KR = len(EXP_T) * len(PAIRS2)  # 30 contraction rows for smooth
PACKW = XU + FREE_LOC       # 585 packed fill width (stationary | moving)

_NC = None                  # compiled Bass module, cached per process
LAST_RESULTS = None         # BassKernelResults of the most recent run (for test.py)


def _ensure_path():
    try:
        import concourse.bass  # noqa: F401
        return
    except ImportError:
        pass
    import sys
    for p in ("/opt/trn_rl_repo", "/root/.axon_site/_ro/trn_rl_repo"):
        if os.path.isdir(p) and p not in sys.path:
            sys.path.insert(0, p)


def _build_nc():
    from contextlib import ExitStack
    from concourse import bass, mybir

    f32 = mybir.dt.float32
    bf16 = mybir.dt.bfloat16
    nc = bass.Bass()

    pack6 = nc.dram_tensor("pack6", [K6, PACKW], bf16, kind="ExternalInput")
    packr = nc.dram_tensor("packr", [KR, PACKW], bf16, kind="ExternalInput")
    # chunk-major packing: [sharp_c0 | smooth_c0 | sharp_c1 | smooth_c1]
    out = nc.dram_tensor("out", [XU, 2 * N_CHUNKS * CHUNK], f32,
                         kind="ExternalOutput")

    ctx = ExitStack()
    with ctx:
        si6 = ctx.enter_context(nc.semaphore("si6"))  # sharp-factor fill
        sir = ctx.enter_context(nc.semaphore("sir"))  # smooth-factor fill
        sp = ctx.enter_context(nc.semaphore("sp"))    # matmul completions
        sa = ctx.enter_context(nc.semaphore("sa"))    # ACT copy completions
        sv = ctx.enter_context(nc.semaphore("sv"))    # DVE copy completions
        ss = ctx.enter_context(nc.semaphore("ss"))    # output DMAs

        t6 = ctx.enter_context(nc.sbuf_tensor("t6", [K6, PACKW], bf16))
        tr = ctx.enter_context(nc.sbuf_tensor("tr", [KR, PACKW], bf16))
        acc6 = [
            ctx.enter_context(nc.psum_tensor(f"acc6_{c}", [XU, CHUNK], f32))
            for c in range(N_CHUNKS)
        ]
        accr = [
            ctx.enter_context(nc.psum_tensor(f"accr_{c}", [XU, CHUNK], f32))
            for c in range(N_CHUNKS)
        ]
        # packed chunk-major result tile
        sbo = ctx.enter_context(
            nc.sbuf_tensor("sbo", [XU, 2 * N_CHUNKS * CHUNK], f32)
        )
        # warmup scratch: never written, data is irrelevant
        tw = ctx.enter_context(nc.sbuf_tensor("tw", [1, 8], bf16))
        accw = ctx.enter_context(nc.psum_tensor("accw", [1, 8], f32))

        # ---- fills: sharp factors on sync (matmuls lead with them), smooth
        # factors on scalar, in parallel
        nc.sync.dma_start(t6[:], pack6[:]).then_inc(si6, 16)
        nc.scalar.dma_start(tr[:], packr[:]).then_inc(sir, 16)

        # ---- PE: a no-wait dummy matmul on scratch data starts the p-state
        # ramp during the fill phase
        nc.tensor.matmul(accw[:], tw[:, 0:1], tw[:, 0:8])

        # sharp chunks then smooth chunks, each into its own PSUM bank.
        # sp ticks: mm6_0=1, mm6_1=2, mmr_0=3, mmr_1=4
        for c in range(N_CHUNKS):
            msl = slice(XU + c * CHUNK, XU + (c + 1) * CHUNK)
            mm = nc.tensor.matmul(acc6[c][:], t6[:, 0:XU], t6[:, msl])
            if c == 0:
                mm._wait_ge(si6, 16)
            mm.then_inc(sp, 1)
        for c in range(N_CHUNKS):
            msl = slice(XU + c * CHUNK, XU + (c + 1) * CHUNK)
            mm = nc.tensor.matmul(accr[c][:], tr[:, 0:XU], tr[:, msl])
            if c == 0:
                mm._wait_ge(sir, 16)
            mm.then_inc(sp, 1)

        # ---- ACT: sharp PSUM -> sbo chunk slots 0 and 2
        for c in range(N_CHUNKS):
            lo = 2 * c * CHUNK
            cp = nc.scalar.copy(sbo[:, lo:lo + CHUNK], acc6[c][:])
            cp._wait_ge(sp, c + 1)
            cp.then_inc(sa, 1)

        # ---- DVE: smooth PSUM -> sbo chunk slots 1 and 3.  The spacer waits
        # observe the ACT copies so each write below needs only one sv wait.
        for c in range(N_CHUNKS):
            lo = (2 * c + 1) * CHUNK
            nc.vector.wait_ge(sa, c + 1)
            cp = nc.vector.tensor_copy(sbo[:, lo:lo + CHUNK], accr[c][:])
            cp._wait_ge(sp, N_CHUNKS + c + 1)
            cp.then_inc(sv, 1)

        # ---- combined per-chunk writes: chunk 0 on sync, chunk 1 on scalar
        d = nc.sync.dma_start(out[:, 0:2 * CHUNK], sbo[:, 0:2 * CHUNK])
        d._wait_ge(sv, 1)
        d.then_inc(ss, 16)
        d = nc.scalar.dma_start(out[:, 2 * CHUNK:4 * CHUNK],
                                sbo[:, 2 * CHUNK:4 * CHUNK])
        d._wait_ge(sv, 2)
        d.then_inc(ss, 16)

        # retire
        nc.sync.wait_ge(ss, 32)
    return nc


def _split3(x):
    """Split f32 vector into 3 bf16 components summing to ~x (2^-24)."""
    import ml_dtypes
    bf = ml_dtypes.bfloat16
    x = x.astype(np.float32)
    x0 = x.astype(bf)
    r1 = x - x0.astype(np.float32)
    x1 = r1.astype(bf)
    x2 = (r1 - x1.astype(np.float32)).astype(bf)
    return x0, x1, x2


def _host_precompute(gridx, gridy, gridz, m):
    """Build the packed bf16 [stationary | moving] factor images over the
    unique [65 x, 65 y, 64 z] block."""
    from math import comb
    import ml_dtypes

    def cc(g):
        return (np.float32(-2.0) * np.cos(np.float32(2.0 * np.pi) * g)
                + np.float32(2.0))

    ccx = cc(np.concatenate([gridx[:m], gridx[-m:]]))[:XU]   # [65] unique
    ccy = cc(np.concatenate([gridy[:m], gridy[-m:]]))[:YU]   # [65] unique
    ccz = cc(gridz[:m])                                      # [64]

    b = (3.0 * ccx.astype(np.float64) + 1.0)                               # [65]
    a = (3.0 * (ccy[:, None].astype(np.float64)
                + ccz[None, :].astype(np.float64))).reshape(-1)            # [4160]

    bf = ml_dtypes.bfloat16
    pack6 = np.zeros((K6, XU + FREE_U), bf)
    packr = np.zeros((KR, XU + FREE_U), bf)

    # sharp: s^6 = sum_k C(6,k) a^k b^(6-k), 3-way bf16 split, p+q <= 2
    r = 0
    for k in range(KPOW):
        wp = _split3((comb(6, k) * b ** (6 - k)).astype(np.float32))
        mp = _split3((a ** k).astype(np.float32))
        for p, q in PAIRS3:
            pack6[r, :XU] = wp[p]
            pack6[r, XU:] = mp[q]
            r += 1

    # smooth: s^-6 ~= sum_n w_n e^(-t_n b) e^(-t_n a), 2-way split, 3 pairs
    r = 0
    for t, w in zip(EXP_T, EXP_W):
        wp = _split3((w * np.exp(-t * b)).astype(np.float32))
        mp = _split3(np.exp(-t * a).astype(np.float32))
        for p, q in PAIRS2:
            packr[r, :XU] = wp[p]
            packr[r, XU:] = mp[q]
            r += 1
    return pack6, packr


def kernel(gridx, gridy, gridz, mode, batchsize):
    _ensure_path()
    global _NC, LAST_RESULTS
    from concourse.bass_utils import run_bass_kernel_spmd

    m = int(mode)
    bsz = int(batchsize)
    assert m == MODE and bsz == BATCH, (m, bsz)

    gridx = np.asarray(gridx, np.float32)
    gridy = np.asarray(gridy, np.float32)
    gridz = np.asarray(gridz, np.float32)

    pack6, packr = _host_precompute(gridx, gridy, gridz, m)

    if _NC is None:
        _NC = _build_nc()

    in_maps = []
    for c in range(N_CORES):
        sl = slice(XU + c * FREE_LOC, XU + (c + 1) * FREE_LOC)
        in_maps.append({
            "pack6": np.concatenate([pack6[:, :XU], pack6[:, sl]], axis=1),
            "packr": np.concatenate([packr[:, :XU], packr[:, sl]], axis=1),
        })
    res = run_bass_kernel_spmd(_NC, in_maps, core_ids=list(range(N_CORES)))
    LAST_RESULTS = res

    # unshard: unpack chunk-major slots, stitch cores, expand mirrors + batch
    sharp_parts, smooth_parts = [], []
    for r in res.results:
        o = r["out"]                      # [65, 1040]
        sharp_parts += [o[:, 0:CHUNK], o[:, 2 * CHUNK:3 * CHUNK]]
        smooth_parts += [o[:, CHUNK:2 * CHUNK], o[:, 3 * CHUNK:4 * CHUNK]]
    sharp_u = np.concatenate(sharp_parts, axis=1).reshape(XU, YU, MODE)
    smooth_u = np.concatenate(smooth_parts, axis=1).reshape(XU, YU, MODE)

    # mirror maps: full index i -> unique index (i if i <= 64 else 128 - i)
    xmap = np.concatenate([np.arange(XU), np.arange(MODE - 1, 0, -1)])
    sharp_plane = sharp_u[xmap][:, xmap, :]           # [128, 128, 64]
    smooth_plane = smooth_u[xmap][:, xmap, :]

    full = (BATCH, 1, TWO_M, TWO_M, MODE)
    smooth = np.broadcast_to(np.ascontiguousarray(smooth_plane)[None, None], full)
    sharp = np.broadcast_to(np.ascontiguousarray(sharp_plane)[None, None], full)
    return (smooth, sharp)


# revision 16
# speedup vs baseline: 1.0331x; 1.0331x over previous
"""Bass/Trainium2 kernel for nn_Epdiff: Hermitian-truncated EPDiff smoothing
filters.

reference:
    cc(g) = -2*cos(2*pi*g) + 2
    s[i,j,k] = 3*(cc(gx)[i] + cc(gy)[j] + cc(gz)[k]) + 1     (gx,gy 2m-band, gz m)
    sharp  = s**6, smooth = s**-6, broadcast to [B, 1, 2m, 2m, m]

Work reduction before any device code runs:
  - The batch axis is a pure broadcast in the reference (jnp.broadcast_to):
    the unique output is one [128, 128, 64] plane per output.
  - cc is even around g=0.5, so the x band is mirror symmetric
    (plane[i] == plane[128-i] for i > 64) and likewise the y band:
    only the [65, 65, 64] corner block is unique (~1/3.9 of the plane).
The host unshard step expands batch with np.broadcast_to (zero copy) and the
x/y mirrors with two cheap np.take index maps (error vs the reference's
directly-evaluated cos values is ~1 ulp of cos, amplified to ~5e-6 relative).

The unique block ([65 partitions (x), 65*64 = 4160 free columns (y,z)]) is
sharded 520 columns per core.  Both outputs are evaluated entirely on the PE
as rank-structured matmuls over separable per-axis factors
(a_f = 3*(cc(gy)+cc(gz)), b_i = 3*cc(gx)+1, s = a + b):
  - sharp:  s^6 = sum_k C(6,k) a^k b^(6-k)           (7 binomial terms)
  - smooth: s^-6 ~= sum_n w_n e^(-t_n a) e^(-t_n b)  (10-node positive
    exponential-sum fit, abs err 1.7e-7 on s in [1,19]; the error metric is
    relative to max|smooth| = 1, so absolute accuracy is what matters)
Each factor is split into bf16 components (3-way / pairs p+q<=2 for sharp ->
42 rows; 2-way / 3 pairs for smooth -> 30 rows) so the bf16 PE reproduces the
f32 products to ~6e-7 / ~3e-6 while streaming 1 column/cycle.  The PE does
the partition broadcast for free: no SBUF broadcast fill, no ACT/DVE power
or reciprocal chain.

Scheduling: the NEFF entry/exit protocol costs ~6.7us + ~1.2us no matter
what, every DMA trigger occupies its queue ~0.6-1.2us plus ~1.2us of queue
kick latency, and a DMA-completion semaphore takes ~0.5-1us to land, so the
kernel minimizes DMA count and spreads the two fast HWDGE rings:
  sync  : sharp-factor fill -> combined chunk-0 output write -> retire
  scalar: smooth-factor fill -> sharp PSUM->SBUF copies (ACT Copy needs no
          table swap; its one-time table load hides under the fills)
          -> combined chunk-1 output write
  tensor: p-state warmup matmul + 4 real matmuls (2 chunks x 2 outputs)
  vector: smooth PSUM->SBUF copies, with spacer waits observing the ACT
          copies so each combined write needs only one semaphore wait
  gpsimd: unused (Q7 triggers are ~700ns and their semaphores lag ~2us)
Results are packed chunk-major in one SBUF tile ([65, 1040] =
sharp_c0|smooth_c0|sharp_c1|smooth_c1) so each write moves 65 x 2080-byte
lines (DMA packet-rate is ~170ns/packet/engine; fewer, bigger packets win);
the host splits the packing during unshard.
"""

import os
import numpy as np

# ---- problem constants (hardcoded per spec) ----
MODE = 64
TWO_M = 2 * MODE            # 128 = full x/y band size
XU = MODE + 1               # 65 unique x rows (partition axis)
YU = MODE + 1               # 65 unique y blocks
FREE_U = YU * MODE          # 4160 unique free columns (y,z)
BATCH = 32
N_CORES = 8
FREE_LOC = FREE_U // N_CORES  # 520 columns per core
CHUNK = FREE_LOC // 2       # 260 columns: fits one PSUM bank, <= 512 moving
N_CHUNKS = 2
KPOW = 7                    # binomial terms k = 0..6
PAIRS3 = ((0, 0), (0, 1), (1, 0), (0, 2), (1, 1), (2, 0))
PAIRS2 = ((0, 0), (0, 1), (1, 0))
K6 = KPOW * len(PAIRS3)     # 42 contraction rows for sharp
# positive exponential-sum nodes/weights for s^-6 on [1, 19]
# (bounded least squares on log-spaced nodes; abs err 1.7e-7)
EXP_T = (0.5, 0.7613351081512586, 1.1592622938073773, 1.7651741676630313,
         2.6877781316870837, 4.092599709149147, 6.231679684369749,
         9.488793052927289, 14.44830257035062, 22.0)
EXP_W = (0.00013164787718447632, 0.00024017744752813375,
         0.010191058868615295, 0.10040649134112098, 1.3406085839989488,
         16.376637870042973, 205.80671073439115, 2549.8737222649515,
         32029.378543336225, 466119.860099446)
KR = len(EXP_T) * len(PAIRS2)  # 30 contraction rows for smooth
PACKW = XU + FREE_LOC       # 585 packed fill width (stationary | moving)

_NC = None                  # compiled Bass module, cached per process
LAST_RESULTS = None         # BassKernelResults of the most recent run (for test.py)


def _ensure_path():
    try:
        import concourse.bass  # noqa: F401
        return
    except ImportError:
        pass
    import sys
    for p in ("/opt/trn_rl_repo", "/root/.axon_site/_ro/trn_rl_repo"):
        if os.path.isdir(p) and p not in sys.path:
            sys.path.insert(0, p)


def _build_nc():
    from contextlib import ExitStack
    from concourse import bass, mybir

    f32 = mybir.dt.float32
    bf16 = mybir.dt.bfloat16
    nc = bass.Bass()

    pack6 = nc.dram_tensor("pack6", [K6, PACKW], bf16, kind="ExternalInput")
    packr = nc.dram_tensor("packr", [KR, PACKW], bf16, kind="ExternalInput")
    # chunk-major packing: [sharp_c0 | smooth_c0 | sharp_c1 | smooth_c1]
    out = nc.dram_tensor("out", [XU, 2 * N_CHUNKS * CHUNK], f32,
                         kind="ExternalOutput")

    ctx = ExitStack()
    with ctx:
        si6 = ctx.enter_context(nc.semaphore("si6"))  # sharp-factor fill
        sir = ctx.enter_context(nc.semaphore("sir"))  # smooth-factor fill
        sp = ctx.enter_context(nc.semaphore("sp"))    # matmul completions
        sa = ctx.enter_context(nc.semaphore("sa"))    # ACT copy completions
        sv = ctx.enter_context(nc.semaphore("sv"))    # DVE copy completions
        ss = ctx.enter_context(nc.semaphore("ss"))    # output DMAs

        t6 = ctx.enter_context(nc.sbuf_tensor("t6", [K6, PACKW], bf16))
        tr = ctx.enter_context(nc.sbuf_tensor("tr", [KR, PACKW], bf16))
        acc6 = [
            ctx.enter_context(nc.psum_tensor(f"acc6_{c}", [XU, CHUNK], f32))
            for c in range(N_CHUNKS)
        ]
        accr = [
            ctx.enter_context(nc.psum_tensor(f"accr_{c}", [XU, CHUNK], f32))
            for c in range(N_CHUNKS)
        ]
        # packed chunk-major result tile
        sbo = ctx.enter_context(
            nc.sbuf_tensor("sbo", [XU, 2 * N_CHUNKS * CHUNK], f32)
        )
        # warmup scratch: never written, data is irrelevant
        tw = ctx.enter_context(nc.sbuf_tensor("tw", [1, 8], bf16))
        accw = ctx.enter_context(nc.psum_tensor("accw", [1, 8], f32))

        # ---- fills: sharp factors on sync (matmuls lead with them), smooth
        # factors on scalar, in parallel
        nc.sync.dma_start(t6[:], pack6[:]).then_inc(si6, 16)
        nc.scalar.dma_start(tr[:], packr[:]).then_inc(sir, 16)

        # ---- PE: a no-wait dummy matmul on scratch data starts the p-state
        # ramp during the fill phase
        nc.tensor.matmul(accw[:], tw[:, 0:1], tw[:, 0:8])

        # sharp chunks then smooth chunks, each into its own PSUM bank.
        # sp ticks: mm6_0=1, mm6_1=2, mmr_0=3, mmr_1=4
        for c in range(N_CHUNKS):
            msl = slice(XU + c * CHUNK, XU + (c + 1) * CHUNK)
            mm = nc.tensor.matmul(acc6[c][:], t6[:, 0:XU], t6[:, msl])
            if c == 0:
                mm._wait_ge(si6, 16)
            mm.then_inc(sp, 1)
        for c in range(N_CHUNKS):
            msl = slice(XU + c * CHUNK, XU + (c + 1) * CHUNK)
            mm = nc.tensor.matmul(accr[c][:], tr[:, 0:XU], tr[:, msl])
            if c == 0:
                mm._wait_ge(sir, 16)
            mm.then_inc(sp, 1)

        # ---- ACT: sharp PSUM -> sbo left half
        for c in range(N_CHUNKS):
            cp = nc.scalar.copy(sbo[:, c * CHUNK:(c + 1) * CHUNK], acc6[c][:])
            cp._wait_ge(sp, c + 1)
            cp.then_inc(sa, 1)

        # ---- DVE: smooth PSUM -> sbo right half
        for c in range(N_CHUNKS):
            lo = (N_CHUNKS + c) * CHUNK
            cp = nc.vector.tensor_copy(sbo[:, lo:lo + CHUNK], accr[c][:])
            cp._wait_ge(sp, N_CHUNKS + c + 1)
            cp.then_inc(sv, 1)

        # ---- output writes on both rings in parallel: sharp half on sync,
        # smooth half on scalar
        d = nc.sync.dma_start(out[:, 0:2 * CHUNK], sbo[:, 0:2 * CHUNK])
        d._wait_ge(sa, N_CHUNKS)
        d.then_inc(ss, 16)
        d = nc.scalar.dma_start(out[:, 2 * CHUNK:4 * CHUNK],
                                sbo[:, 2 * CHUNK:4 * CHUNK])
        d._wait_ge(sv, N_CHUNKS)
        d.then_inc(ss, 16)

        # retire
        nc.sync.wait_ge(ss, 32)
    return nc


def _split3(x):
    """Split f32 vector into 3 bf16 components summing to ~x (2^-24)."""
    import ml_dtypes
    bf = ml_dtypes.bfloat16
    x = x.astype(np.float32)
    x0 = x.astype(bf)
    r1 = x - x0.astype(np.float32)
    x1 = r1.astype(bf)
    x2 = (r1 - x1.astype(np.float32)).astype(bf)
    return x0, x1, x2


def _host_precompute(gridx, gridy, gridz, m):
    """Build the packed bf16 [stationary | moving] factor images over the
    unique [65 x, 65 y, 64 z] block."""
    from math import comb
    import ml_dtypes

    def cc(g):
        return (np.float32(-2.0) * np.cos(np.float32(2.0 * np.pi) * g)
                + np.float32(2.0))

    ccx = cc(np.concatenate([gridx[:m], gridx[-m:]]))[:XU]   # [65] unique
    ccy = cc(np.concatenate([gridy[:m], gridy[-m:]]))[:YU]   # [65] unique
    ccz = cc(gridz[:m])                                      # [64]

    b = (3.0 * ccx.astype(np.float64) + 1.0)                               # [65]
    a = (3.0 * (ccy[:, None].astype(np.float64)
                + ccz[None, :].astype(np.float64))).reshape(-1)            # [4160]

    bf = ml_dtypes.bfloat16
    pack6 = np.zeros((K6, XU + FREE_U), bf)
    packr = np.zeros((KR, XU + FREE_U), bf)

    # sharp: s^6 = sum_k C(6,k) a^k b^(6-k), 3-way bf16 split, p+q <= 2
    r = 0
    for k in range(KPOW):
        wp = _split3((comb(6, k) * b ** (6 - k)).astype(np.float32))
        mp = _split3((a ** k).astype(np.float32))
        for p, q in PAIRS3:
            pack6[r, :XU] = wp[p]
            pack6[r, XU:] = mp[q]
            r += 1

    # smooth: s^-6 ~= sum_n w_n e^(-t_n b) e^(-t_n a), 2-way split, 3 pairs
    r = 0
    for t, w in zip(EXP_T, EXP_W):
        wp = _split3((w * np.exp(-t * b)).astype(np.float32))
        mp = _split3(np.exp(-t * a).astype(np.float32))
        for p, q in PAIRS2:
            packr[r, :XU] = wp[p]
            packr[r, XU:] = mp[q]
            r += 1
    return pack6, packr


def kernel(gridx, gridy, gridz, mode, batchsize):
    _ensure_path()
    global _NC, LAST_RESULTS
    from concourse.bass_utils import run_bass_kernel_spmd

    m = int(mode)
    bsz = int(batchsize)
    assert m == MODE and bsz == BATCH, (m, bsz)

    gridx = np.asarray(gridx, np.float32)
    gridy = np.asarray(gridy, np.float32)
    gridz = np.asarray(gridz, np.float32)

    pack6, packr = _host_precompute(gridx, gridy, gridz, m)

    if _NC is None:
        _NC = _build_nc()

    in_maps = []
    for c in range(N_CORES):
        sl = slice(XU + c * FREE_LOC, XU + (c + 1) * FREE_LOC)
        in_maps.append({
            "pack6": np.concatenate([pack6[:, :XU], pack6[:, sl]], axis=1),
            "packr": np.concatenate([packr[:, :XU], packr[:, sl]], axis=1),
        })
    res = run_bass_kernel_spmd(_NC, in_maps, core_ids=list(range(N_CORES)))
    LAST_RESULTS = res

    # unshard: unpack chunk-major slots, stitch cores, expand mirrors + batch
    sharp_parts, smooth_parts = [], []
    for r in res.results:
        o = r["out"]                      # [65, 1040]
        sharp_parts += [o[:, 0:CHUNK], o[:, 2 * CHUNK:3 * CHUNK]]
        smooth_parts += [o[:, CHUNK:2 * CHUNK], o[:, 3 * CHUNK:4 * CHUNK]]
    sharp_u = np.concatenate(sharp_parts, axis=1).reshape(XU, YU, MODE)
    smooth_u = np.concatenate(smooth_parts, axis=1).reshape(XU, YU, MODE)

    # mirror maps: full index i -> unique index (i if i <= 64 else 128 - i)
    xmap = np.concatenate([np.arange(XU), np.arange(MODE - 1, 0, -1)])
    sharp_plane = sharp_u[xmap][:, xmap, :]           # [128, 128, 64]
    smooth_plane = smooth_u[xmap][:, xmap, :]

    full = (BATCH, 1, TWO_M, TWO_M, MODE)
    smooth = np.broadcast_to(np.ascontiguousarray(smooth_plane)[None, None], full)
    sharp = np.broadcast_to(np.ascontiguousarray(sharp_plane)[None, None], full)
    return (smooth, sharp)


# revision 18
# speedup vs baseline: 1.0673x; 1.0331x over previous
"""Bass/Trainium2 kernel for nn_Epdiff: Hermitian-truncated EPDiff smoothing
filters.

reference:
    cc(g) = -2*cos(2*pi*g) + 2
    s[i,j,k] = 3*(cc(gx)[i] + cc(gy)[j] + cc(gz)[k]) + 1     (gx,gy 2m-band, gz m)
    sharp  = s**6, smooth = s**-6, broadcast to [B, 1, 2m, 2m, m]

Work reduction before any device code runs:
  - The batch axis is a pure broadcast in the reference (jnp.broadcast_to):
    the unique output is one [128, 128, 64] plane per output.
  - cc is even around g=0.5, so the x band is mirror symmetric
    (plane[i] == plane[128-i] for i > 64) and likewise the y band:
    only the [65, 65, 64] corner block is unique (~1/3.9 of the plane).
The host unshard step expands batch with np.broadcast_to (zero copy) and the
x/y mirrors with two cheap np.take index maps (error vs the reference's
directly-evaluated cos values is ~1 ulp of cos, amplified to ~5e-6 relative).

The unique block ([65 partitions (x), 65*64 = 4160 free columns (y,z)]) is
sharded 520 columns per core.  Both outputs are evaluated entirely on the PE
as rank-structured matmuls over separable per-axis factors
(a_f = 3*(cc(gy)+cc(gz)), b_i = 3*cc(gx)+1, s = a + b):
  - sharp:  s^6 = sum_k C(6,k) a^k b^(6-k)           (7 binomial terms)
  - smooth: s^-6 ~= sum_n w_n e^(-t_n a) e^(-t_n b)  (10-node positive
    exponential-sum fit, abs err 1.7e-7 on s in [1,19]; the error metric is
    relative to max|smooth| = 1, so absolute accuracy is what matters)
Each factor is split into bf16 components (3-way / pairs p+q<=2 for sharp ->
42 rows; 2-way / 3 pairs for smooth -> 30 rows) so the bf16 PE reproduces the
f32 products to ~6e-7 / ~3e-6 while streaming 1 column/cycle.  The PE does
the partition broadcast for free: no SBUF broadcast fill, no ACT/DVE power
or reciprocal chain.

Scheduling: the NEFF entry/exit protocol costs ~6.7us + ~1.2us no matter
what, every DMA trigger occupies its queue ~0.6-1.2us plus ~1.2us of queue
kick latency, and a DMA-completion semaphore takes ~0.5-1us to land, so the
kernel minimizes DMA count and spreads the two fast HWDGE rings:
  sync  : sharp-factor fill -> combined chunk-0 output write -> retire
  scalar: smooth-factor fill -> sharp PSUM->SBUF copies (ACT Copy needs no
          table swap; its one-time table load hides under the fills)
          -> combined chunk-1 output write
  tensor: p-state warmup matmul + 4 real matmuls (2 chunks x 2 outputs)
  vector: smooth PSUM->SBUF copies, with spacer waits observing the ACT
          copies so each combined write needs only one semaphore wait
  gpsimd: unused (Q7 triggers are ~700ns and their semaphores lag ~2us)
Results are packed in one SBUF tile ([65, 1040] = sharp half | smooth half)
so each write moves 65 x 2080-byte lines (DMA packet-rate is ~170ns/packet/
engine; fewer, bigger packets win); the host splits the packing during
unshard.
"""

import os
import numpy as np

# ---- problem constants (hardcoded per spec) ----
MODE = 64
TWO_M = 2 * MODE            # 128 = full x/y band size
XU = MODE + 1               # 65 unique x rows (partition axis)
YU = MODE + 1               # 65 unique y blocks
FREE_U = YU * MODE          # 4160 unique free columns (y,z)
BATCH = 32
N_CORES = 8
FREE_LOC = FREE_U // N_CORES  # 520 columns per core
CHUNK = FREE_LOC // 2       # 260 columns: fits one PSUM bank, <= 512 moving
N_CHUNKS = 2
KPOW = 7                    # binomial terms k = 0..6
PAIRS3 = ((0, 0), (0, 1), (1, 0), (0, 2), (1, 1), (2, 0))
PAIRS2 = ((0, 0), (0, 1), (1, 0))
K6 = KPOW * len(PAIRS3)     # 42 contraction rows for sharp
# positive exponential-sum nodes/weights for s^-6 on [1, 19]
# (bounded least squares on log-spaced nodes; abs err 1.7e-7)
EXP_T = (0.5, 0.7613351081512586, 1.1592622938073773, 1.7651741676630313,
         2.6877781316870837, 4.092599709149147, 6.231679684369749,
         9.488793052927289, 14.44830257035062, 22.0)
EXP_W = (0.00013164787718447632, 0.00024017744752813375,
         0.010191058868615295, 0.10040649134112098, 1.3406085839989488,
         16.376637870042973, 205.80671073439115, 2549.8737222649515,
         32029.378543336225, 466119.860099446)
KR = len(EXP_T) * len(PAIRS2)  # 30 contraction rows for smooth
PACKW = XU + FREE_LOC       # 585 packed fill width (stationary | moving)

_NC = None                  # compiled Bass module, cached per process
LAST_RESULTS = None         # BassKernelResults of the most recent run (for test.py)


def _ensure_path():
    try:
        import concourse.bass  # noqa: F401
        return
    except ImportError:
        pass
    import sys
    for p in ("/opt/trn_rl_repo", "/root/.axon_site/_ro/trn_rl_repo"):
        if os.path.isdir(p) and p not in sys.path:
            sys.path.insert(0, p)


def _build_nc():
    from contextlib import ExitStack
    from concourse import bass, mybir

    f32 = mybir.dt.float32
    bf16 = mybir.dt.bfloat16
    nc = bass.Bass()

    pack6 = nc.dram_tensor("pack6", [K6, PACKW], bf16, kind="ExternalInput")
    packr = nc.dram_tensor("packr", [KR, PACKW], bf16, kind="ExternalInput")
    # chunk-major packing: [sharp_c0 | smooth_c0 | sharp_c1 | smooth_c1]
    out = nc.dram_tensor("out", [XU, 2 * N_CHUNKS * CHUNK], f32,
                         kind="ExternalOutput")

    ctx = ExitStack()
    with ctx:
        si6 = ctx.enter_context(nc.semaphore("si6"))  # sharp-factor fill
        sir = ctx.enter_context(nc.semaphore("sir"))  # smooth-factor fill
        sp = ctx.enter_context(nc.semaphore("sp"))    # matmul completions
        sa = ctx.enter_context(nc.semaphore("sa"))    # ACT copy completions
        sv = ctx.enter_context(nc.semaphore("sv"))    # DVE copy completions
        ss = ctx.enter_context(nc.semaphore("ss"))    # output DMAs

        t6 = ctx.enter_context(nc.sbuf_tensor("t6", [K6, PACKW], bf16))
        tr = ctx.enter_context(nc.sbuf_tensor("tr", [KR, PACKW], bf16))
        acc6 = [
            ctx.enter_context(nc.psum_tensor(f"acc6_{c}", [XU, CHUNK], f32))
            for c in range(N_CHUNKS)
        ]
        accr = [
            ctx.enter_context(nc.psum_tensor(f"accr_{c}", [XU, CHUNK], f32))
            for c in range(N_CHUNKS)
        ]
        # packed chunk-major result tile
        sbo = ctx.enter_context(
            nc.sbuf_tensor("sbo", [XU, 2 * N_CHUNKS * CHUNK], f32)
        )
        # warmup scratch: never written, data is irrelevant
        tw = ctx.enter_context(nc.sbuf_tensor("tw", [1, 8], bf16))
        accw = ctx.enter_context(nc.psum_tensor("accw", [1, 8], f32))

        # ---- fills: sharp factors on sync (matmuls lead with them), smooth
        # factors on scalar, in parallel
        nc.sync.dma_start(t6[:], pack6[:]).then_inc(si6, 16)
        nc.scalar.dma_start(tr[:], packr[:]).then_inc(sir, 16)

        # ---- PE: a no-wait dummy matmul on scratch data starts the p-state
        # ramp during the fill phase
        nc.tensor.matmul(accw[:], tw[:, 0:1], tw[:, 0:8])

        # sharp chunks then smooth chunks, each into its own PSUM bank.
        # sp ticks: mm6_0=1, mm6_1=2, mmr_0=3, mmr_1=4
        for c in range(N_CHUNKS):
            msl = slice(XU + c * CHUNK, XU + (c + 1) * CHUNK)
            mm = nc.tensor.matmul(acc6[c][:], t6[:, 0:XU], t6[:, msl])
            if c == 0:
                mm._wait_ge(si6, 16)
            mm.then_inc(sp, 1)
        for c in range(N_CHUNKS):
            msl = slice(XU + c * CHUNK, XU + (c + 1) * CHUNK)
            mm = nc.tensor.matmul(accr[c][:], tr[:, 0:XU], tr[:, msl])
            if c == 0:
                mm._wait_ge(sir, 16)
            mm.then_inc(sp, 1)

        # ---- ACT: sharp PSUM -> sbo left half
        for c in range(N_CHUNKS):
            cp = nc.scalar.copy(sbo[:, c * CHUNK:(c + 1) * CHUNK], acc6[c][:])
            cp._wait_ge(sp, c + 1)
            cp.then_inc(sa, 1)

        # ---- DVE: smooth PSUM -> sbo right half
        for c in range(N_CHUNKS):
            lo = (N_CHUNKS + c) * CHUNK
            cp = nc.vector.tensor_copy(sbo[:, lo:lo + CHUNK], accr[c][:])
            cp._wait_ge(sp, N_CHUNKS + c + 1)
            cp.then_inc(sv, 1)

        # ---- output writes on both rings in parallel: sharp half on sync,
        # smooth half on scalar
        d = nc.sync.dma_start(out[:, 0:2 * CHUNK], sbo[:, 0:2 * CHUNK])
        d._wait_ge(sa, N_CHUNKS)
        d.then_inc(ss, 16)
        d = nc.scalar.dma_start(out[:, 2 * CHUNK:4 * CHUNK],
                                sbo[:, 2 * CHUNK:4 * CHUNK])
        d._wait_ge(sv, N_CHUNKS)
        d.then_inc(ss, 16)

        # retire
        nc.sync.wait_ge(ss, 32)
    return nc


def _split3(x):
    """Split f32 vector into 3 bf16 components summing to ~x (2^-24)."""
    import ml_dtypes
    bf = ml_dtypes.bfloat16
    x = x.astype(np.float32)
    x0 = x.astype(bf)
    r1 = x - x0.astype(np.float32)
    x1 = r1.astype(bf)
    x2 = (r1 - x1.astype(np.float32)).astype(bf)
    return x0, x1, x2


def _host_precompute(gridx, gridy, gridz, m):
    """Build the packed bf16 [stationary | moving] factor images over the
    unique [65 x, 65 y, 64 z] block."""
    from math import comb
    import ml_dtypes

    def cc(g):
        return (np.float32(-2.0) * np.cos(np.float32(2.0 * np.pi) * g)
                + np.float32(2.0))

    ccx = cc(np.concatenate([gridx[:m], gridx[-m:]]))[:XU]   # [65] unique
    ccy = cc(np.concatenate([gridy[:m], gridy[-m:]]))[:YU]   # [65] unique
    ccz = cc(gridz[:m])                                      # [64]

    b = (3.0 * ccx.astype(np.float64) + 1.0)                               # [65]
    a = (3.0 * (ccy[:, None].astype(np.float64)
                + ccz[None, :].astype(np.float64))).reshape(-1)            # [4160]

    bf = ml_dtypes.bfloat16
    pack6 = np.zeros((K6, XU + FREE_U), bf)
    packr = np.zeros((KR, XU + FREE_U), bf)

    # sharp: s^6 = sum_k C(6,k) a^k b^(6-k), 3-way bf16 split, p+q <= 2
    r = 0
    for k in range(KPOW):
        wp = _split3((comb(6, k) * b ** (6 - k)).astype(np.float32))
        mp = _split3((a ** k).astype(np.float32))
        for p, q in PAIRS3:
            pack6[r, :XU] = wp[p]
            pack6[r, XU:] = mp[q]
            r += 1

    # smooth: s^-6 ~= sum_n w_n e^(-t_n b) e^(-t_n a), 2-way split, 3 pairs
    r = 0
    for t, w in zip(EXP_T, EXP_W):
        wp = _split3((w * np.exp(-t * b)).astype(np.float32))
        mp = _split3(np.exp(-t * a).astype(np.float32))
        for p, q in PAIRS2:
            packr[r, :XU] = wp[p]
            packr[r, XU:] = mp[q]
            r += 1
    return pack6, packr


def kernel(gridx, gridy, gridz, mode, batchsize):
    _ensure_path()
    global _NC, LAST_RESULTS
    from concourse.bass_utils import run_bass_kernel_spmd

    m = int(mode)
    bsz = int(batchsize)
    assert m == MODE and bsz == BATCH, (m, bsz)

    gridx = np.asarray(gridx, np.float32)
    gridy = np.asarray(gridy, np.float32)
    gridz = np.asarray(gridz, np.float32)

    pack6, packr = _host_precompute(gridx, gridy, gridz, m)

    if _NC is None:
        _NC = _build_nc()

    in_maps = []
    for c in range(N_CORES):
        sl = slice(XU + c * FREE_LOC, XU + (c + 1) * FREE_LOC)
        in_maps.append({
            "pack6": np.concatenate([pack6[:, :XU], pack6[:, sl]], axis=1),
            "packr": np.concatenate([packr[:, :XU], packr[:, sl]], axis=1),
        })
    res = run_bass_kernel_spmd(_NC, in_maps, core_ids=list(range(N_CORES)))
    LAST_RESULTS = res

    # unshard: unpack the halves, stitch cores, expand mirrors + batch
    sharp_parts, smooth_parts = [], []
    for r in res.results:
        o = r["out"]                      # [65, 1040] = sharp half | smooth half
        sharp_parts.append(o[:, 0:2 * CHUNK])
        smooth_parts.append(o[:, 2 * CHUNK:4 * CHUNK])
    sharp_u = np.concatenate(sharp_parts, axis=1).reshape(XU, YU, MODE)
    smooth_u = np.concatenate(smooth_parts, axis=1).reshape(XU, YU, MODE)

    # mirror maps: full index i -> unique index (i if i <= 64 else 128 - i)
    xmap = np.concatenate([np.arange(XU), np.arange(MODE - 1, 0, -1)])
    sharp_plane = sharp_u[xmap][:, xmap, :]           # [128, 128, 64]
    smooth_plane = smooth_u[xmap][:, xmap, :]

    full = (BATCH, 1, TWO_M, TWO_M, MODE)
    smooth = np.broadcast_to(np.ascontiguousarray(smooth_plane)[None, None], full)
    sharp = np.broadcast_to(np.ascontiguousarray(sharp_plane)[None, None], full)
    return (smooth, sharp)
